# revision 37
# baseline (speedup 1.0000x reference)
"""BipartiteGCN message-passing kernel for 8 TRN2 NeuronCores.

Math:  out = D_c^{-1/2} A^T D_r^{-1/2} (x @ W) + b
where A[s, d] = multiplicity of edge (gene s, drug d), s, d in [0, 4000).

Strategy (dst-window sharding):
  - Core c owns drug (dst) window [512c, 512c+512).  Edges are sharded to
    cores by dst window and bucketed by (gene window 128, dst subwindow 128)
    (host-side layout only; all arithmetic happens on device).
  - Each core builds its dense count stripe A_c [4096 genes x 512 drugs] in
    SBUF with 128x128 one-hot x one-hot PE matmuls (fp16, 1 cycle/row), one
    [128,512] PSUM tile per gene window (4 dst-subwindow column groups x 3
    chunks).  One-hot builds are split evenly between DVE and GPSIMD.
  - xW is computed half per core in float32r (1 cycle/row): HBM-pair cores
    (2k, 2k+1) each compute one 2048-gene half and exchange through a
    pair-Shared DRAM buffer; the row_deg AllGather doubles as the pair sync
    (a WAR data-dependency gates the read-back).  The parity register comes
    from a per-core input tensor, keeping the SPMD program rank-independent.
  - row_deg partials come free from the Act-engine PSUM->SBUF copy of A
    (accum_out); exchanged via a small AllGather (128KB) and summed locally.
    col_deg = ones^T @ A on the PE (local).  Two of the four xW blocks are
    schedule-hinted into the AllGather window so the PE stays busy.
  - out = g * ((f*A)^T @ xW) + bias, GEMMs in bf16; f folded into A in a
    DVE burst, g/bias fused into the dt-outer output copy path.
"""

import sys

if "/opt/trn_rl_repo" not in sys.path:
    sys.path.insert(0, "/opt/trn_rl_repo")

import numpy as np

import concourse.bass as bass  # noqa: F401
import concourse.mybir as mybir
from concourse import bacc, tile

CORES = 8
DWIN = 512              # dst (drug) window per core
ND = 4000               # number of drugs
GD = 4096               # padded gene dim (src < 4000)
IC = 1024
OC = 512
GT = GD // 128          # 32 gene windows
DT = DWIN // 128        # 4 dst subwindows per core
CPB = 3                 # chunks per (gwin, dwin) bucket
NCH = GT * DT * CPB     # 384 chunks per core
NSLOT = NCH * 128       # 49152 edge slots per core

F32 = mybir.dt.float32
F32R = mybir.dt.float32r
F16 = mybir.dt.float16
BF16 = mybir.dt.bfloat16
AX = mybir.AxisListType
OP = mybir.AluOpType
ACT = mybir.ActivationFunctionType


def build_nc(debug_outputs=False):
    nc = bacc.Bacc(
        None,
        target_bir_lowering=False,
        debug=False,
        num_devices=CORES,
    )

    xT = nc.dram_tensor("xT", [IC, GD // 2], F32R, kind="ExternalInput")
    par = nc.dram_tensor("par", [1, 1], mybir.dt.int32, kind="ExternalInput")
    w = nc.dram_tensor("w", [IC, OC], F32R, kind="ExternalInput")
    brep = nc.dram_tensor("brep", [128, OC], F32, kind="ExternalInput")
    i128 = nc.dram_tensor("i128", [128, 128], F16, kind="ExternalInput")
    sloc = nc.dram_tensor("sloc", [128, NCH], F32, kind="ExternalInput")
    dloc = nc.dram_tensor("dloc", [128, NCH], F32, kind="ExternalInput")
    out = nc.dram_tensor("out", [DWIN, OC], F32, kind="ExternalOutput")

    rdl = nc.dram_tensor("rdl", [128, GT], F32)            # rowdeg partial
    rds = nc.dram_tensor("rds", [CORES * 128, GT], F32, addr_space="Shared")
    cdl = nc.dram_tensor("cdl", [1, OC], F32)              # coldeg bounce
    # pair-shared xW buffer: cores (2k, 2k+1) share HBM; each writes the
    # half it computed (by parity) and reads the full tensor back after the
    # AllGather (which doubles as the pair sync)
    xwsh = nc.dram_tensor("xwsh", [GD, OC], BF16, addr_space="Shared")

    Adbg = None
    if debug_outputs:
        Adbg = nc.dram_tensor("Adbg", [GD, OC], F32, kind="ExternalOutput")

    with tile.TileContext(nc) as tc:
        with (
            tc.tile_pool(name="const", bufs=1) as cpool,
            tc.tile_pool(name="work", bufs=3) as wpool,
            tc.tile_pool(name="oh", bufs=24) as ohpool,
            tc.tile_pool(name="apool", bufs=GT) as apool,
            tc.tile_pool(name="xwpool", bufs=GT) as xwpool,
            tc.tile_pool(name="psA", bufs=2, space="PSUM") as psA,
            tc.tile_pool(name="psB", bufs=4, space="PSUM") as psB,
            tc.tile_pool(name="psC", bufs=1, space="PSUM") as psC,
        ):
            # ---- constants ----
            ones_sb = cpool.tile([128, 1], BF16)
            nc.vector.memset(ones_sb[:], 1.0)
            i128_sb = cpool.tile([128, 128], F16)
            nc.sync.dma_start(i128_sb[:], i128[:])
            bias_sb = cpool.tile([128, OC], F32)
            nc.sync.dma_start(bias_sb[:], brep[:])
            sloc_sb = cpool.tile([128, NCH], F32)
            nc.sync.dma_start(sloc_sb[:], sloc[:])
            dloc_sb = cpool.tile([128, NCH], F32)
            nc.sync.dma_start(dloc_sb[:], dloc[:])
            # w tiles stay resident (f32r)
            w_sb = []
            for kt in range(8):
                wt = cpool.tile([128, OC], F32R, name=f"w{kt}")
                nc.sync.dma_start(wt[:], w[kt * 128:(kt + 1) * 128, :])
                w_sb.append(wt)

            a_sb = [None] * GT
            xw_sb = [None] * GT
            rd_sb = cpool.tile([128, GT], F32)
            pcd = psC.tile([1, OC], F32, tag="cd")

            # ---- A-build (gene window t) + full local xW, interleaved ----
            for t in range(GT):
                pa = psA.tile([128, OC], F32, tag="bld", name=f"pa{t}")
                for dwin in range(DT):
                    for i in range(CPB):
                        c = (t * DT + dwin) * CPB + i
                        loh = ohpool.tile([128, 128], F16, tag="loh",
                                          name=f"loh{c}")
                        roh = ohpool.tile([128, 128], F16, tag="roh",
                                          name=f"roh{c}")
                        # split one-hot builds evenly between DVE and GPSIMD
                        eng_l = nc.vector if (c % 2 == 0) else nc.gpsimd
                        eng_r = nc.gpsimd if (c % 2 == 0) else nc.vector
                        eng_l.tensor_scalar(
                            out=loh[:], in0=i128_sb[:],
                            scalar1=sloc_sb[:, c:c + 1], scalar2=None,
                            op0=OP.is_equal,
                        )
                        eng_r.tensor_scalar(
                            out=roh[:], in0=i128_sb[:],
                            scalar1=dloc_sb[:, c:c + 1], scalar2=None,
                            op0=OP.is_equal,
                        )
                        nc.tensor.matmul(
                            pa[:, dwin * 128:(dwin + 1) * 128],
                            loh[:], roh[:],
                            start=(i == 0), stop=(i == CPB - 1),
                            skip_group_check=True,
                        )
                # PSUM -> SBUF copy (cast bf16) + free rowdeg partial
                a_t = apool.tile([128, OC], BF16, tag="A", name=f"a{t}")
                nc.scalar.activation(
                    a_t[:], pa[:], ACT.Copy,
                    accum_out=rd_sb[:, t:t + 1],
                )
                a_sb[t] = a_t
                if debug_outputs:
                    dbg = wpool.tile([128, OC], F32, tag="dbg", name=f"dbg{t}")
                    nc.vector.tensor_copy(dbg[:], a_t[:])
                    nc.sync.dma_start(Adbg[t * 128:(t + 1) * 128, :], dbg[:])
                # coldeg accumulation: pcd += ones^T @ A_t   (bf16, 1cyc)
                nc.tensor.matmul(
                    pcd[:], ones_sb[:], a_t[:],
                    start=(t == 0), stop=(t == GT - 1),
                )
            # parity register (which gene half this core computes)
            par_sb = cpool.tile([1, 1], mybir.dt.int32)
            nc.sync.dma_start(par_sb[:], par[:])
            preg = nc.sync.alloc_register("parr")
            nc.sync.load(preg, par_sb[0:1, 0:1])
            pv = nc.sync.snap(preg)
            HALF = (GD // 2) * OC  # elements per xwsh half

            def emit_b_block(i_b):
                # local xW block i_b of this core's gene half; written to
                # pair-shared DRAM rows [2048*parity + 512*i_b, ...+512)
                pb = [psB.tile([128, OC], F32, tag="bp",
                               name=f"pb{i_b}_{j}") for j in range(4)]
                for kt in range(8):
                    xt_t = wpool.tile([128, OC], F32R, tag="xT",
                                      name=f"xt{i_b}_{kt}")
                    nc.sync.dma_start(
                        xt_t[:],
                        xT[kt * 128:(kt + 1) * 128,
                           i_b * 512:(i_b + 1) * 512],
                    )
                    for j in range(4):
                        nc.tensor.matmul(
                            pb[j][:],
                            xt_t[:, j * 128:(j + 1) * 128],
                            w_sb[kt][:],
                            start=(kt == 0), stop=(kt == 7),
                        )
                from concourse.ap import AP as APcls
                for j in range(4):
                    stg = xwpool.tile([128, OC], BF16, tag="STG",
                                      name=f"stg{i_b}_{j}")
                    nc.scalar.activation(stg[:], pb[j][:], ACT.Copy)
                    r0 = (i_b * 4 + j) * 128
                    ap0 = xwsh[r0:r0 + 128, :]
                    dyn = APcls(tensor=ap0.tensor,
                                offset=ap0.offset + pv * HALF,
                                ap=ap0.ap, dep_tracking_offset=ap0.offset)
                    nc.sync.dma_start(dyn, stg[:])

            # xW half: 2 blocks early, 2 held back for the AllGather window
            for i_b in range(2):
                emit_b_block(i_b)

            # ---- rowdeg partial exchange (AllGather + local sum) ----
            nc.sync.dma_start(rdl[:], rd_sb[:])
            nc.gpsimd.collective_compute(
                "AllGather",
                OP.bypass,
                replica_groups=[list(range(CORES))],
                ins=[rdl[:].opt()],
                outs=[rds[:].opt()],
            )

            # last 2 xW blocks: scheduled late so the PE chews on them
            # while the AllGather is in flight
            for k, i_b in enumerate(range(2, 4)):
                with tc.tile_wait_until((43 + 12 * k) * 1e-6):
                    emit_b_block(i_b)

            deg = cpool.tile([128, GT], F32)
            parts = []
            for r in range(CORES):
                pt = wpool.tile([128, GT], F32, tag="rdp", bufs=CORES,
                                name=f"rdp{r}")
                nc.sync.dma_start(pt[:], rds[r * 128:(r + 1) * 128, :])
                parts.append(pt)
            # gate value derived from the AllGather output: any op using
            # dgate is data-ordered after the collective, which implies the
            # pair partner finished writing its xW half to xwsh
            dgate = cpool.tile([1, 1], F32)
            nc.vector.tensor_scalar(
                out=dgate[:], in0=parts[0][0:1, 0:1], scalar1=0.0,
                scalar2=None, op0=OP.mult)
            trash = cpool.tile([1, 1], F32)
            for t in range(GT):
                xw_t = xwpool.tile([128, OC], BF16, tag="XW", name=f"xw{t}")
                # pre-reader depending on dgate: the subsequent DMA write into
                # xw_t picks up a WAR dependency, so the read-back cannot
                # start before the AllGather completed
                nc.vector.memset(xw_t[0:1, 0:1], 0.0)
                nc.vector.tensor_scalar(
                    out=trash[:], in0=xw_t[0:1, 0:1],
                    scalar1=dgate[0:1, 0:1], scalar2=None, op0=OP.mult)
                nc.sync.dma_start(xw_t[:], xwsh[t * 128:(t + 1) * 128, :])
                xw_sb[t] = xw_t
            nc.vector.tensor_tensor(
                out=deg[:], in0=parts[0][:], in1=parts[1][:], op=OP.add)
            for r in range(2, CORES):
                nc.vector.tensor_tensor(
                    out=deg[:], in0=deg[:], in1=parts[r][:], op=OP.add)

            # f = (deg>0)/sqrt(max(deg,1))
            t1 = cpool.tile([128, GT], F32)
            nc.vector.tensor_scalar(
                out=t1[:], in0=deg[:], scalar1=1.0, scalar2=None, op0=OP.max)
            nc.scalar.sqrt(t1[:], t1[:])
            nc.vector.reciprocal(t1[:], t1[:])
            fmask = cpool.tile([128, GT], F32)
            nc.vector.tensor_scalar(
                out=fmask[:], in0=deg[:], scalar1=0.5, scalar2=None,
                op0=OP.is_gt)
            f_sb = cpool.tile([128, GT], F32)
            nc.vector.tensor_tensor(
                out=f_sb[:], in0=t1[:], in1=fmask[:], op=OP.mult)

            # ---- coldeg -> g  ([1,512] -> [128,4], drug dt*128+p on part p)
            cd_row = cpool.tile([1, OC], F32)
            nc.vector.tensor_copy(cd_row[:], pcd[:])
            nc.sync.dma_start(cdl[:], cd_row[:])
            cd_sb = cpool.tile([128, DT], F32)
            for kq in range(DT):
                nc.sync.dma_start(
                    cd_sb[:, kq:kq + 1], cdl[0:1, kq * 128:(kq + 1) * 128])
            g1 = cpool.tile([128, DT], F32)
            nc.vector.tensor_scalar(
                out=g1[:], in0=cd_sb[:], scalar1=1.0, scalar2=None, op0=OP.max)
            nc.scalar.sqrt(g1[:], g1[:])
            nc.vector.reciprocal(g1[:], g1[:])
            gmask = cpool.tile([128, DT], F32)
            nc.vector.tensor_scalar(
                out=gmask[:], in0=cd_sb[:], scalar1=0.5, scalar2=None,
                op0=OP.is_gt)
            g_sb = cpool.tile([128, DT], F32)
            nc.vector.tensor_tensor(
                out=g_sb[:], in0=g1[:], in1=gmask[:], op=OP.mult)

            # ---- F: out = (f*A)^T @ xW  (bf16, accumulated over windows)
            # f-scale burst first so the PE never waits per-tile on the DVE;
            # dt-outer matmul order so each output quarter finishes (and its
            # g-scale/bias/store runs) while the next quarter still computes
            for t in range(GT):
                nc.vector.tensor_scalar(
                    out=a_sb[t][:], in0=a_sb[t][:],
                    scalar1=f_sb[:, t:t + 1], scalar2=None, op0=OP.mult)
            po = [psB.tile([128, OC], F32, tag="bp", name=f"po{j}")
                  for j in range(4)]
            for dt_i in range(4):
                for t in range(GT):
                    nc.tensor.matmul(
                        po[dt_i][:],
                        a_sb[t][:, dt_i * 128:(dt_i + 1) * 128],
                        xw_sb[t][:],
                        start=(t == 0), stop=(t == GT - 1),
                    )
                og = wpool.tile([128, OC], F32, tag="og", name=f"og{dt_i}")
                nc.scalar.activation(
                    og[:], po[dt_i][:], ACT.Copy,
                    scale=g_sb[:, dt_i:dt_i + 1],
                )
                nc.vector.tensor_tensor(
                    out=og[:], in0=og[:], in1=bias_sb[:], op=OP.add)
                nc.sync.dma_start(out[dt_i * 128:(dt_i + 1) * 128, :], og[:])

    nc.finalize()
    return nc


def make_in_maps(x, weight, bias, edge_index):
    """Host-side sharding/layout only: no arithmetic on tensor values."""
    x = np.asarray(x, dtype=np.float32)
    weight = np.ascontiguousarray(np.asarray(weight, dtype=np.float32))
    bias = np.asarray(bias, dtype=np.float32)
    ei = np.asarray(edge_index)
    s_all = ei[0].astype(np.int64)
    d_all = ei[1].astype(np.int64)
    assert s_all.min() >= 0 and s_all.max() < ND, "src ids out of range"
    assert d_all.min() >= 0 and d_all.max() < ND, "dst ids out of range"

    brep = np.ascontiguousarray(
        np.tile(bias[None, :], (128, 1)).astype(np.float32))
    i128 = np.ascontiguousarray(
        np.tile(np.arange(128, dtype=np.float16)[None, :], (128, 1)))

    x4 = np.zeros((GD, IC), dtype=np.float32)
    x4[:ND] = x[:ND]
    xT_halves = [
        np.ascontiguousarray(x4[:GD // 2].T),
        np.ascontiguousarray(x4[GD // 2:].T),
    ]

    core_of = d_all >> 9
    in_maps = []
    for c in range(CORES):
        m = core_of == c
        s = s_all[m]
        dl = d_all[m] - c * DWIN

        # bucket by (gene window, dst subwindow); CPB chunks per bucket
        b = (s >> 7) * DT + (dl >> 7)
        o = np.argsort(b, kind="stable")
        s_o, dl_o, b_o = s[o], dl[o], b[o]
        cnt = np.bincount(b_o, minlength=GT * DT)
        assert cnt.max() <= CPB * 128, f"bucket overflow: {cnt.max()}"

        sl_lin = np.full(NSLOT, -1.0, dtype=np.float32)
        dl_lin = np.full(NSLOT, -1.0, dtype=np.float32)
        pos = 0
        for bb in range(GT * DT):
            n = int(cnt[bb])
            base = bb * CPB * 128
            sl_lin[base:base + n] = (s_o[pos:pos + n] & 127).astype(np.float32)
            dl_lin[base:base + n] = (dl_o[pos:pos + n] & 127).astype(np.float32)
            pos += n

        sloc_t = np.ascontiguousarray(sl_lin.reshape(NCH, 128).T)
        dloc_t = np.ascontiguousarray(dl_lin.reshape(NCH, 128).T)

        in_maps.append(
            {
                "xT": xT_halves[c & 1],
                "par": np.array([[c & 1]], dtype=np.int32),
                "w": weight,
                "brep": brep,
                "i128": i128,
                "sloc": sloc_t,
                "dloc": dloc_t,
            }
        )
    return in_maps


_NC = None


def _get_nc():
    global _NC
    if _NC is None:
        _NC = build_nc()
    return _NC


def kernel(x, weight, bias, edge_index, **run_kwargs):
    from concourse.bass_utils import run_bass_kernel_spmd

    nc = _get_nc()
    in_maps = make_in_maps(x, weight, bias, edge_index)
    res = run_bass_kernel_spmd(nc, in_maps, core_ids=list(range(CORES)),
                               **run_kwargs)
    outs = res.results if hasattr(res, "results") else res
    full = np.empty((ND, OC), dtype=np.float32)
    for c in range(CORES):
        n = min(DWIN, ND - c * DWIN)
        full[c * DWIN:c * DWIN + n] = outs[c]["out"][:n]
    if run_kwargs:
        return full, res
    return full
